# revision 3
# baseline (speedup 1.0000x reference)
"""Trainium2 Bass kernel for nn_MultiHeadAttnBlock (GN + 4-head attn + proj + residual).

Problem (hardcoded shapes): x_kv [1,256,64,64] f32, 4 heads, head_dim 64,
n = 64*64 = 4096 tokens, GroupNorm(32 groups, eps=1e-6).

Sharding: query-parallel over 8 cores, K/V replicated. The reference's
torch-faithful output reshape (`[b,n,H,hd].reshape(b,c,h,w)`) reinterprets
memory so that proj-conv input channel c at pixel p is the attention output
of token 16*c + p//256, channel p%256. Hence core `cid` owns tokens
{n : n mod 16 in {2*cid, 2*cid+1}} and its output pixels are the contiguous
block [512*cid, 512*(cid+1)). A host-side column permutation puts each
core's 512 tokens first, so all 8 cores run one identical program (pure
SPMD, no collectives, no dynamic addressing).

Per-core pipeline (all fp32):
  load x (perm'd) -> GroupNorm stats (bn_stats + group-mask matmuls)
  -> h = norm(x) -> K (channel-major, full), V (token-major, full, with a
     ones column appended per head), Q (channel-major, own 512 tokens,
     pre-scaled by hd^-0.5)
  -> per head: S^T = K^T Q (keys on partitions), P^T = exp(S^T) (no max
     subtraction; scores are O(6) so fp32 exp is safe), O^T/sums via one
     accumulating matmul (ones column makes row 64 the softmax denom),
     normalize by broadcasted reciprocal row-sums (outer-product matmul)
  -> un-reshape transposes -> proj matmul + bias + residual -> out.
"""

import sys

sys.path.insert(0, "/opt/trn_rl_repo")

import numpy as np

import concourse.bass as bass
import concourse.bacc as bacc
import concourse.mybir as mybir
import concourse.tile as tile
from concourse.bass_utils import run_bass_kernel_spmd

F32 = mybir.dt.float32
AF = mybir.ActivationFunctionType
ALU = mybir.AluOpType

C = 256          # channels
N = 4096         # tokens (h*w)
NS = 512         # tokens per core (query slice)
H = 4            # heads
HD = 64          # head dim
G = 32           # groupnorm groups
GPC = C // G     # channels per group = 8
P = 128          # partitions
CT = C // P      # channel tiles = 2
NCORES = 8
EPS = 1e-6
SCALE = HD ** -0.5  # 0.125
SB = 2           # j-tiles per exp batch (psum_s spans SB PSUM banks)
SKEW = 2         # exp -> O-matmul software-pipeline depth (blocks)
F32R = mybir.dt.float32r  # PE-reduced-precision fp32: 4x matmul throughput


_CACHE = {}


def _emit_o(nc, psum_o, v_sb, pt, jp, h):
    for b in range(SB):
        jt = jp * SB + b
        nc.tensor.matmul(
            psum_o[0:HD + 1, :],
            v_sb[:, jt, h * (HD + 1):(h + 1) * (HD + 1)],
            pt[:, b, :],
            start=(jt == 0), stop=(jt == N // P - 1),
        )


def _write_trivial(nc, outp, out_d, xres_sb):
    for t in range(CT):
        y_sb = outp.tile([P, NS], F32, name="ysb", tag="ysb")
        nc.vector.tensor_copy(out=y_sb, in_=xres_sb[:, t, :])
        nc.sync.dma_start(out=out_d[t * P:(t + 1) * P, :], in_=y_sb)


def build_nc(reps=1, stop_after=None):
    nc = bacc.Bacc("TRN2", target_bir_lowering=False, debug=False, num_devices=NCORES)

    # ---- I/O ----
    x_d = nc.dram_tensor("x", [C, N], F32R, kind="ExternalInput")
    xres_d = nc.dram_tensor("xres", [C, NS], F32, kind="ExternalInput")
    w_d = {}
    b_d = {}
    for nm in ("wq", "wk", "wv", "wp"):
        w_d[nm] = nc.dram_tensor(nm, [C, C], F32, kind="ExternalInput")
    for nm in ("bq", "bk", "bv", "bp", "gamma", "beta"):
        b_d[nm] = nc.dram_tensor(nm, [C, 1], F32, kind="ExternalInput")
    ident_d = nc.dram_tensor("ident", [P, P], F32, kind="ExternalInput")
    # mask8[p, g] = 1/8 if p//8 == g else 0   (channel -> group averaging)
    mask8_d = nc.dram_tensor("mask8", [P, 16], F32, kind="ExternalInput")
    # mask16T[g, p] = 1 if p//8 == g else 0   (group -> channel broadcast)
    mask16t_d = nc.dram_tensor("mask16t", [16, P], F32, kind="ExternalInput")
    ones64_d = nc.dram_tensor("ones64", [1, HD], F32, kind="ExternalInput")
    out_d = nc.dram_tensor("out", [C, NS], F32, kind="ExternalOutput")

    with tile.TileContext(nc) as tc:
        with (
            tc.tile_pool(name="persist", bufs=1) as pp,
            tc.tile_pool(name="wraw", bufs=2) as wraw_pool,
            tc.tile_pool(name="pt", bufs=7) as pt_pool,
            tc.tile_pool(name="small", bufs=4) as sm,
            tc.tile_pool(name="outp", bufs=2) as outp,
            tc.tile_pool(name="ps", bufs=2, space="PSUM") as ps_pool,
            tc.tile_pool(name="po", bufs=2, space="PSUM") as po_pool,
            tc.tile_pool(name="pm", bufs=2, space="PSUM") as pm_pool,
        ):
            # ---------- constants ----------
            ident = pp.tile([P, P], F32, name="ident", tag="ident")
            nc.sync.dma_start(out=ident, in_=ident_d[:, :])
            mask8 = pp.tile([P, 16], F32, name="mask8", tag="mask8")
            nc.sync.dma_start(out=mask8, in_=mask8_d[:, :])
            mask16t = pp.tile([16, P], F32, name="mask16t", tag="mask16t")
            nc.sync.dma_start(out=mask16t, in_=mask16t_d[:, :])
            ones64 = pp.tile([1, HD], F32, name="ones64", tag="ones64")
            nc.sync.dma_start(out=ones64, in_=ones64_d[:, :])
            warm = sm.tile([1, 1], F32, name="warm", tag="warm")
            nc.scalar.activation(out=warm, in_=ones64[:, 0:1], func=AF.Exp)

            bias_sb = {}
            for nm in ("bq", "bk", "bv", "bp", "gamma", "beta"):
                for t in range(CT):
                    b_t = pp.tile([P, 1], F32, name=f"{nm}{t}", tag=f"{nm}{t}")
                    nc.sync.dma_start(out=b_t, in_=b_d[nm][t * P:(t + 1) * P, :])
                    bias_sb[(nm, t)] = b_t

            for _rep in range(reps):
                # ---------- load + transpose weights ----------
                wT = {}  # wT[nm][ct] : [128 (in_c), 256 (out_c)]
                for nm in ("wq", "wk", "wv", "wp"):
                    wT[nm] = [pp.tile([P, C], F32R, name=f"{nm}T{ct}", tag=f"{nm}T{ct}") for ct in range(CT)]
                    for ot in range(CT):  # row tile of W = out-channel tile
                        wr = wraw_pool.tile([P, C], F32, name="wraw", tag="wraw")
                        weng = nc.gpsimd if ot % 2 else nc.sync
                        weng.dma_start(
                            out=wr, in_=w_d[nm][ot * P:(ot + 1) * P, :]
                        )
                        for ct in range(CT):  # col block = in-channel block
                            ps_t = pm_pool.tile([P, P], F32, name="pm", tag="pm")
                            nc.tensor.transpose(
                                ps_t, wr[:, ct * P:(ct + 1) * P], ident
                            )
                            nc.vector.tensor_copy(
                                out=wT[nm][ct][:, ot * P:(ot + 1) * P], in_=ps_t
                            )

                if stop_after == "load":
                    _write_trivial(nc, outp, out_d, xres_sb)
                    continue
                # ---------- load x (chunked for DMA/compute overlap) ----------
                x_sb = []
                for t in range(CT):
                    xt = pp.tile([P, N], F32R, name=f"x{t}", tag=f"x{t}")
                    for ch in range(8):
                        eng = nc.sync if (ch + t) % 2 == 0 else nc.gpsimd
                        eng.dma_start(
                            out=xt[:, ch * 512:(ch + 1) * 512],
                            in_=x_d[t * P:(t + 1) * P, ch * 512:(ch + 1) * 512],
                        )
                    x_sb.append(xt)
                xres_sb = pp.tile([P, CT, NS], F32, name="xres", tag="xres")
                for t in range(CT):
                    nc.gpsimd.dma_start(
                        out=xres_sb[:, t, :], in_=xres_d[t * P:(t + 1) * P, :]
                    )

                # ---------- GroupNorm statistics ----------
                stat2 = []  # [128, 2] per tile: (mean_c, E[x^2]_c)
                for t in range(CT):
                    stats = sm.tile([P, 8, 6], F32, name=f"bnst{t}", tag=f"bnst{t}")
                    for sg in range(8):
                        nc.vector.bn_stats(
                            out=stats[:, sg, :],
                            in_=x_sb[t][:, sg * 512:(sg + 1) * 512].bitcast(F32),
                        )
                    mv = sm.tile([P, 2], F32, name=f"mv{t}", tag=f"mv{t}")
                    nc.vector.bn_aggr(out=mv, in_=stats)
                    nc.vector.scalar_tensor_tensor(
                        out=mv[:, 1:2], in0=mv[:, 0:1], scalar=mv[:, 0:1],
                        in1=mv[:, 1:2], op0=ALU.mult, op1=ALU.add,
                    )
                    stat2.append(mv)

                # group aggregation: psum_g[16, 4] cols (mean_t0, E2_t0, mean_t1, E2_t1)
                psum_g = pm_pool.tile([16, 2 * CT], F32, name="pm", tag="pm")
                for t in range(CT):
                    nc.tensor.matmul(
                        psum_g[:, 2 * t:2 * t + 2], mask8, stat2[t],
                        start=True, stop=True,
                    )
                gsb = sm.tile([16, 2, CT], F32, name="gsb", tag="gsb")
                nc.vector.tensor_copy(
                    out=gsb, in_=psum_g.rearrange("p (t s) -> p s t", s=2)
                )
                gmean = gsb[:, 0, :]   # [16, CT]
                gE2 = gsb[:, 1, :]     # [16, CT]
                gmsq = sm.tile([16, CT], F32, name="gmsq", tag="gmsq")
                nc.vector.tensor_mul(gmsq, gmean, gmean)
                gvar = sm.tile([16, CT], F32, name="gvar", tag="gvar")
                nc.vector.tensor_sub(gvar, gE2, gmsq)
                eps16 = sm.tile([16, 1], F32, name="eps16", tag="eps16")
                nc.vector.memset(eps16, EPS)
                glog = sm.tile([16, CT], F32, name="glog", tag="glog")
                nc.scalar.activation(out=glog, in_=gvar, func=AF.Ln, bias=eps16)
                # gstats[16, {mean,rstd}, CT]; rstd = exp(-0.5*ln(var+eps))
                gstats = sm.tile([16, 2, CT], F32, name="gstats", tag="gstats")
                nc.scalar.activation(out=gstats[:, 1, :], in_=glog, func=AF.Exp,
                                     scale=-0.5)
                nc.vector.tensor_copy(out=gstats[:, 0, :], in_=gmean)

                # A = rstd*gamma, B = beta - mean*A (per channel), then fold
                # GroupNorm into the QKV weights: W <- W diag(A), bias += W@B
                A_sb, B_sb = [], []
                for t in range(CT):
                    psum_ch = pm_pool.tile([P, 2], F32, name="pm", tag="pm")
                    nc.tensor.matmul(
                        psum_ch, mask16t, gstats[:, :, t], start=True, stop=True
                    )
                    A_t = sm.tile([P, 1], F32, name=f"A{t}", tag=f"A{t}")
                    nc.vector.tensor_mul(A_t, psum_ch[:, 1:2], bias_sb[("gamma", t)])
                    tmp_t = sm.tile([P, 1], F32, name=f"mt{t}", tag=f"mt{t}")
                    nc.vector.tensor_mul(tmp_t, psum_ch[:, 0:1], A_t)
                    B_t = sm.tile([P, 1], F32R, name=f"B{t}", tag=f"B{t}")
                    nc.vector.tensor_sub(B_t, bias_sb[("beta", t)], tmp_t)
                    A_sb.append(A_t)
                    B_sb.append(B_t)
                # bias corrections W@B (use unfolded weights; folds below wait)
                kb, qb, wv_corr = [], [], []
                for nm, dst in (("wq", qb), ("wk", kb), ("wv", wv_corr)):
                    for ot in range(CT):
                        psum_bc = pm_pool.tile([P, 1], F32, name="pm", tag="pm")
                        for ct in range(CT):
                            nc.tensor.matmul(
                                psum_bc,
                                wT[nm][ct][:, ot * P:(ot + 1) * P].bitcast(F32),
                                B_sb[ct].bitcast(F32),
                                start=(ct == 0), stop=(ct == CT - 1),
                            )
                        b_t = sm.tile([P, 1], F32, name=f"bc{nm}{ot}",
                                      tag=f"bc{nm}{ot}")
                        if nm == "wk":
                            nc.vector.tensor_add(b_t, psum_bc, bias_sb[("bk", ot)])
                        elif nm == "wq":
                            nc.vector.tensor_scalar(
                                out=b_t, in0=psum_bc,
                                scalar1=bias_sb[("bq", ot)], scalar2=SCALE,
                                op0=ALU.add, op1=ALU.mult,
                            )
                        else:
                            nc.vector.tensor_add(b_t, psum_bc, bias_sb[("bv", ot)])
                        dst.append(b_t)
                # fold A (and hd^-0.5 for Q) into the weight columns
                AQ_sb = []
                for t in range(CT):
                    aq = sm.tile([P, 1], F32, name=f"AQ{t}", tag=f"AQ{t}")
                    nc.vector.tensor_scalar_mul(aq, A_sb[t], SCALE)
                    AQ_sb.append(aq)
                for ct in range(CT):
                    nc.vector.tensor_scalar_mul(wT["wq"][ct], wT["wq"][ct], AQ_sb[ct])
                    nc.vector.tensor_scalar_mul(wT["wk"][ct], wT["wk"][ct], A_sb[ct])
                for ct in range(CT):
                    nc.vector.tensor_scalar_mul(wT["wv"][ct], wT["wv"][ct], A_sb[ct])

                if stop_after == "gn":
                    _write_trivial(nc, outp, out_d, xres_sb)
                    continue
                # ---------- Q (channel-major, own 512 tokens, pre-scaled) ----------
                q_cm = [pp.tile([P, NS], F32R, name=f"q{t}", tag=f"q{t}") for t in range(CT)]
                for ot in range(CT):
                    psum_q = pm_pool.tile([P, NS], F32, name="pm", tag="pm")
                    for ct in range(CT):
                        nc.tensor.matmul(
                            psum_q,
                            wT["wq"][ct][:, ot * P:(ot + 1) * P],
                            x_sb[ct][:, 0:NS],
                            start=(ct == 0), stop=(ct == CT - 1),
                        )
                    nc.vector.tensor_scalar(
                        out=q_cm[ot], in0=psum_q,
                        scalar1=qb[ot], scalar2=None,
                        op0=ALU.add,
                    )

                if stop_after == "conv":
                    _write_trivial(nc, outp, out_d, xres_sb)
                    continue

                # ---------- K/V storage; interleaved conv+attention ----------
                # pass 0: produce K/V blocks just-in-time + attention heads 0,1
                # pass 1: attention heads 2,3 (K/V already resident)
                k_cm = [pp.tile([P, N], F32R, name=f"k{t}", tag=f"k{t}") for t in range(CT)]
                v_sb = pp.tile([P, N // P, H * (HD + 1)], F32R, name="vtm", tag="vtm")
                v4 = v_sb.rearrange("p j (h e) -> p j h e", e=HD + 1)
                nc.vector.tensor_scalar(
                    out=v4[:, :, :, HD:], in0=ident[:, 0:N // P * H],
                    scalar1=0.0, scalar2=1.0, op0=ALU.mult, op1=ALU.add,
                )
                attn_cm = [pp.tile([P, NS], F32, name=f"attn{t}", tag=f"attn{t}") for t in range(CT)]
                attnT = [pp.tile([P, 2, C], F32R, name=f"attnT{b}", tag=f"attnT{b}") for b in range(CT)]
                BLK = SB * P  # 256 tokens per block
                for pas in range(2):
                    heads = (0, 1) if pas == 0 else (2, 3)
                    po_h = {h: po_pool.tile([P, NS], F32, name="po", tag="po")
                            for h in heads}
                    pend = {h: [] for h in heads}
                    pt_by_h = {}
                    def emit_k(kjp):
                        for ot in range(CT):
                            psum_k = pm_pool.tile([P, BLK], F32, name="pm", tag="pm")
                            for ct in range(CT):
                                nc.tensor.matmul(
                                    psum_k,
                                    wT["wk"][ct][:, ot * P:(ot + 1) * P],
                                    x_sb[ct][:, kjp * BLK:(kjp + 1) * BLK],
                                    start=(ct == 0), stop=(ct == CT - 1),
                                )
                            nc.vector.tensor_scalar(
                                out=k_cm[ot][:, kjp * BLK:(kjp + 1) * BLK],
                                in0=psum_k,
                                scalar1=kb[ot], scalar2=None,
                                op0=ALU.add,
                            )

                    if pas == 0:
                        emit_k(0)
                    for jp in range(N // BLK):
                        for h in heads:
                            t, r0 = h // 2, (h % 2) * HD
                            psum_s = ps_pool.tile([P, SB, NS], F32, name="ps", tag="ps")
                            for b in range(SB):
                                jt = jp * SB + b
                                nc.tensor.matmul(
                                    psum_s[:, b, :],
                                    k_cm[t][r0:r0 + HD, jt * P:(jt + 1) * P],
                                    q_cm[t][r0:r0 + HD, :],
                                    start=True, stop=True,
                                )
                            pt = pt_pool.tile([P, SB, NS], F32R, name="pt", tag="pt")
                            nc.scalar.activation(out=pt, in_=psum_s, func=AF.Exp)
                            pt_by_h[h] = pt
                        if pas == 0:
                            if jp + 1 < N // BLK:
                                emit_k(jp + 1)
                            for b in range(SB):
                                jt = jp * SB + b
                                psum_v = pm_pool.tile([P, C], F32, name="pm", tag="pm")
                                for ct in range(CT):
                                    nc.tensor.matmul(
                                        psum_v,
                                        x_sb[ct][:, jt * P:(jt + 1) * P],
                                        wT["wv"][ct],
                                        start=(ct == 0), stop=(ct == CT - 1),
                                    )
                                nc.vector.tensor_copy(
                                    out=v4[:, jt, :, 0:HD],
                                    in_=psum_v.rearrange("p (h d) -> p h d", d=HD),
                                )
                        for h in heads:
                            pend[h].append((jp, pt_by_h[h]))
                            if len(pend[h]) > SKEW:
                                ojp, opt = pend[h].pop(0)
                                _emit_o(nc, po_h[h], v_sb, opt, ojp, h)
                    for h in heads:
                        for ojp, opt in pend[h]:
                            _emit_o(nc, po_h[h], v_sb, opt, ojp, h)
                        pend[h] = []
                        t, r0 = h // 2, (h % 2) * HD
                        psum_o = po_h[h]
                        rs = sm.tile([1, NS], F32, name="rs", tag="rs")
                        nc.vector.reciprocal(out=rs, in_=psum_o[HD:HD + 1, :])
                        bsb = sm.tile([HD, NS], F32, name="bsb", tag="bsb")
                        nc.gpsimd.partition_broadcast(bsb, rs)
                        nc.vector.tensor_mul(
                            attn_cm[t][r0:r0 + HD, :], psum_o[0:HD, :], bsb
                        )
                        nc.vector.tensor_scalar(
                            out=attn_cm[t][r0:r0 + HD, :],
                            in0=attn_cm[t][r0:r0 + HD, :],
                            scalar1=wv_corr[t][r0:r0 + HD, :], scalar2=None,
                            op0=ALU.add,
                        )
                    if stop_after is None:
                        # un-reshape transposes + proj for attn row-tile `pas`
                        # (its two heads just finished; overlaps next pass)
                        a = pas
                        for s in range(2):
                            for b in range(CT):
                                ps_t = pm_pool.tile([P, P], F32, name="pm", tag="pm")
                                nc.tensor.transpose(
                                    ps_t,
                                    attn_cm[a][:, s * 256 + b * P:s * 256 + (b + 1) * P],
                                    ident,
                                )
                                nc.vector.tensor_copy(
                                    out=attnT[b][:, s, a * P:(a + 1) * P], in_=ps_t
                                )

                if stop_after == "attn":
                    _write_trivial(nc, outp, out_d, xres_sb)
                    continue
                # ---------- proj + bias + residual ----------
                for s in range(2):
                    for ot in range(CT):
                        psum_y = pm_pool.tile([P, C], F32, name="pm", tag="pm")
                        for ct in range(CT):
                            nc.tensor.matmul(
                                psum_y,
                                wT["wp"][ct][:, ot * P:(ot + 1) * P],
                                attnT[ct][:, s, :],
                                start=(ct == 0), stop=(ct == CT - 1),
                            )
                        y_sb = outp.tile([P, C], F32, name="ysb", tag="ysb")
                        c0 = s * 256
                        nc.vector.scalar_tensor_tensor(
                            out=y_sb, in0=psum_y,
                            scalar=bias_sb[("bp", ot)],
                            in1=xres_sb[:, ot, c0:c0 + C],
                            op0=ALU.add, op1=ALU.add,
                        )
                        nc.sync.dma_start(
                            out=out_d[ot * P:(ot + 1) * P, c0:c0 + C],
                            in_=y_sb,
                        )
    nc.compile()
    return nc


def _host_constants():
    ident = np.eye(P, dtype=np.float32)
    mask8 = np.zeros((P, 16), dtype=np.float32)
    mask8[np.arange(P), np.arange(P) // GPC] = 1.0 / GPC
    mask16t = np.zeros((16, P), dtype=np.float32)
    mask16t[np.arange(P) // GPC, np.arange(P)] = 1.0
    ones64 = np.ones((1, HD), dtype=np.float32)
    return ident, mask8, mask16t, ones64


def make_in_maps(x_kv, gn_gamma, gn_beta, Wq, bq, Wk, bk, Wv, bv, Wp, bp):
    x2 = np.ascontiguousarray(np.asarray(x_kv, dtype=np.float32).reshape(C, N))
    ident, mask8, mask16t, ones64 = _host_constants()

    common = {
        "wq": np.ascontiguousarray(np.asarray(Wq, np.float32)),
        "wk": np.ascontiguousarray(np.asarray(Wk, np.float32)),
        "wv": np.ascontiguousarray(np.asarray(Wv, np.float32)),
        "wp": np.ascontiguousarray(np.asarray(Wp, np.float32)),
        "bq": np.asarray(bq, np.float32).reshape(C, 1).copy(),
        "bk": np.asarray(bk, np.float32).reshape(C, 1).copy(),
        "bv": np.asarray(bv, np.float32).reshape(C, 1).copy(),
        "bp": np.asarray(bp, np.float32).reshape(C, 1).copy(),
        "gamma": np.asarray(gn_gamma, np.float32).reshape(C, 1).copy(),
        "beta": np.asarray(gn_beta, np.float32).reshape(C, 1).copy(),
        "ident": ident,
        "mask8": mask8,
        "mask16t": mask16t,
        "ones64": ones64,
    }

    in_maps = []
    for cid in range(NCORES):
        own = np.concatenate(
            [np.arange(2 * cid, N, 16), np.arange(2 * cid + 1, N, 16)]
        )
        rest = np.setdiff1d(np.arange(N), own)
        perm = np.concatenate([own, rest])
        m = dict(common)
        m["x"] = np.ascontiguousarray(x2[:, perm])
        m["xres"] = np.ascontiguousarray(x2[:, NS * cid:NS * (cid + 1)])
        in_maps.append(m)
    return in_maps


def kernel(x_kv, gn_gamma, gn_beta, Wq, bq, Wk, bk, Wv, bv, Wp, bp, **run_kwargs):
    if "nc" not in _CACHE:
        _CACHE["nc"] = build_nc()
    nc = _CACHE["nc"]

    in_maps = make_in_maps(
        x_kv, gn_gamma, gn_beta, Wq, bq, Wk, bk, Wv, bv, Wp, bp
    )

    res = run_bass_kernel_spmd(
        nc, in_maps, core_ids=list(range(NCORES)), **run_kwargs
    )
    y = np.empty((C, N), dtype=np.float32)
    for cid in range(NCORES):
        y[:, NS * cid:NS * (cid + 1)] = res.results[cid]["out"]
    _CACHE["last_results"] = res
    return y.reshape(1, C, 64, 64)



# revision 53
# speedup vs baseline: 1.5460x; 1.5460x over previous
"""Trainium2 Bass kernel for nn_MultiHeadAttnBlock (GN + 4-head attn + proj + residual).

Problem (hardcoded shapes): x_kv [1,256,64,64] f32, 4 heads, head_dim 64,
n = 64*64 = 4096 tokens, GroupNorm(32 groups, eps=1e-6).

Sharding: query-parallel over 8 cores, K/V replicated. The reference's
torch-faithful output reshape (`[b,n,H,hd].reshape(b,c,h,w)`) reinterprets
memory so that proj-conv input channel c at pixel p is the attention output
of token 16*c + p//256, channel p%256. Hence core `cid` owns tokens
{n : n mod 16 in {2*cid, 2*cid+1}} and its output pixels are the contiguous
block [512*cid, 512*(cid+1)). A host-side column permutation puts each
core's 512 tokens first, so all 8 cores run one identical program (pure
SPMD, no collectives, no dynamic addressing).

The per-core critical resource is the Activation engine: softmax needs
exp on 4096 keys x 512 queries x 4 heads = 8.4M elements (~66us at
1 elem/lane/cycle @1.2GHz). Everything else is arranged to hide under it:
  - x ships as bf16 (half the HBM traffic; f32 residual ships separately
    so the output's dominant term stays exact), weights fold GN and run
    as bf16 on the PE.
  - K bias is dropped entirely (softmax is invariant to per-query score
    shifts); V-bias/GN corrections and the attention-output correction
    fold into the proj bias (exact algebra, no extra elementwise passes).
  - K/Q/P(=exp S)/V are stored fp8e4; S^T = K^T Q and O^T = V^T P run as
    fp8 DoubleRow matmuls (2x / 4x PE throughput). exp outputs are
    pre-scaled by 1/16 (bias = -ln16) to sit comfortably in fp8 range;
    numerator and denominator scale together so softmax is unchanged.
  - exp writes fp8 directly; the row-of-ones column in V yields softmax
    denominators from the same accumulating O matmul.
  - GroupNorm rsqrt runs on the DVE (bitcast-Newton) so the Act engine
    loads exactly one activation table (Exp) and does nothing but exp.
  - PSUM->SBUF copies for K/V split across DVE and GpSimd(Pool).
"""

import sys

sys.path.insert(0, "/opt/trn_rl_repo")

import numpy as np
import ml_dtypes

import concourse.bass as bass
import concourse.bacc as bacc
import concourse.mybir as mybir
import concourse.tile as tile
from concourse.bass_utils import run_bass_kernel_spmd

F32 = mybir.dt.float32
F32R = mybir.dt.float32r
BF16 = mybir.dt.bfloat16
F8 = mybir.dt.float8e4
I32 = mybir.dt.int32
AF = mybir.ActivationFunctionType
ALU = mybir.AluOpType
DR = mybir.MatmulPerfMode.DoubleRow

C = 256          # channels
N = 4096         # tokens (h*w)
NS = 512         # tokens per core (query slice)
H = 4            # heads
HD = 64          # head dim
G = 32           # groupnorm groups
GPC = C // G     # channels per group = 8
P = 128          # partitions
CT = C // P      # channel tiles = 2
NCORES = 8
EPS = 1e-6
SCALE = HD ** -0.5  # 0.125
SB = 2           # key-blocks per exp batch / DoubleRow pair
SKEW = 2         # exp -> O-matmul software-pipeline depth (pt tiles)
NBIAS = -float(np.log(16.0))  # exp output pre-scale 1/16 (fp8 headroom)
HDP = 68         # per-head V pitch: 64 values + ones col + pad (dual-fp8
                 # Ldweights wants even/4-aligned weight geometry)
RSQRT_MAGIC = 0x5F3759DF

_CACHE = {}


def _write_trivial(nc, outp, out_d, xres_sb):
    for t in range(CT):
        y_sb = outp.tile([P, NS], F32, name="ysb", tag="ysb")
        nc.vector.tensor_copy(out=y_sb, in_=xres_sb[:, t, :])
        nc.sync.dma_start(out=out_d[t * P:(t + 1) * P, :], in_=y_sb)


def build_nc(reps=1, stop_after=None):
    nc = bacc.Bacc("TRN2", target_bir_lowering=False, debug=False, num_devices=NCORES)

    # ---- I/O ----
    x_d = nc.dram_tensor("x", [C, N], BF16, kind="ExternalInput")
    xres_d = nc.dram_tensor("xres", [C, NS], F32, kind="ExternalInput")
    w_d = {}
    for nm in ("wq", "wk", "wv", "wp"):
        w_d[nm] = nc.dram_tensor(nm, [C, C], F32, kind="ExternalInput")
    # packed per-channel vectors: cols = (bq, bv, bp) per channel row;
    # rows 0..127 additionally carry cols 3:5 = gamma (tile0, tile1) and
    # cols 5:7 = beta (tile0, tile1)
    biasp_d = nc.dram_tensor("biasp", [C, 8], F32, kind="ExternalInput")
    ident_d = nc.dram_tensor("ident", [P, P], F32, kind="ExternalInput")
    # mask8[p, g] = 1/8 if p//8 == g else 0   (channel -> group averaging)
    mask8_d = nc.dram_tensor("mask8", [P, 16], F32, kind="ExternalInput")
    # mask16T[g, p] = 1 if p//8 == g else 0   (group -> channel broadcast)
    mask16t_d = nc.dram_tensor("mask16t", [16, P], F32, kind="ExternalInput")
    out_d = nc.dram_tensor("out", [C, NS], F32, kind="ExternalOutput")
    BIAS_COL = {"bq": 0, "bv": 1, "bp": 2}

    with tile.TileContext(nc) as tc:
        with (
            tc.tile_pool(name="persist", bufs=1) as pp,
            tc.tile_pool(name="wraw", bufs=2) as wraw_pool,
            tc.tile_pool(name="pt", bufs=8) as pt_pool,
            tc.tile_pool(name="small", bufs=4) as sm,
            tc.tile_pool(name="outp", bufs=4) as outp,
            tc.tile_pool(name="ps", bufs=2, space="PSUM") as ps_pool,
            tc.tile_pool(name="po", bufs=2, space="PSUM") as po_pool,
            tc.tile_pool(name="pm", bufs=2, space="PSUM") as pm_pool,
        ):
            # ---------- constants ----------
            # warm the Exp act table immediately (no DMA dependencies)
            nbias = pp.tile([P, 1], F32, name="nbias", tag="nbias")
            nc.vector.memset(nbias, NBIAS)
            warm = sm.tile([1, 1], F32, name="warm", tag="warm")
            nc.scalar.activation(out=warm, in_=nbias[0:1, :], func=AF.Exp,
                                 bias=nbias[0:1, :])
            ident = pp.tile([P, P], F32, name="ident", tag="ident")
            nc.sync.dma_start(out=ident, in_=ident_d[:, :])
            mask8 = pp.tile([P, 16], F32, name="mask8", tag="mask8")
            nc.sync.dma_start(out=mask8, in_=mask8_d[:, :])
            mask16t = pp.tile([16, P], F32, name="mask16t", tag="mask16t")
            nc.sync.dma_start(out=mask16t, in_=mask16t_d[:, :])

            biasp = [pp.tile([P, 8], F32, name=f"biasp{t}", tag=f"biasp{t}")
                     for t in range(CT)]
            for t in range(CT):
                nc.sync.dma_start(out=biasp[t],
                                  in_=biasp_d[t * P:(t + 1) * P, :])
            bias_sb = {
                (nm, t): biasp[t][:, c:c + 1]
                for nm, c in BIAS_COL.items() for t in range(CT)
            }

            # persistent attention operands (values rewritten every rep;
            # zero/ones slots initialized once)
            # q8e: pair slot 0 = Q, slot 1 = 0  (for even key blocks)
            # q8o: pair slot 0 = 0, slot 1 = Q  (for odd key blocks)
            q8e = [pp.tile([P, 2, NS], F8, name=f"q8e{t}", tag=f"q8e{t}") for t in range(CT)]
            q8o = [pp.tile([P, 2, NS], F8, name=f"q8o{t}", tag=f"q8o{t}") for t in range(CT)]
            for t in range(CT):
                nc.vector.memset(q8e[t][:, 1, :], 0.0)
                nc.vector.memset(q8o[t][:, 0, :], 0.0)
            # k8[t][:, s, jp*128 + i] = K channel row, key block 2*jp+s, key i
            k8 = [pp.tile([P, 2, N // 2], F8, name=f"k8{t}", tag=f"k8{t}") for t in range(CT)]
            # v8: token-major V with a ones column per head (softmax denom)
            v8 = pp.tile([P, N // P, H * HDP], F8, name="vtm", tag="vtm")
            v4 = v8.rearrange("p j (h e) -> p j h e", e=HDP)
            nc.vector.tensor_scalar(
                out=v4[:, :, :, HD:HD + 1], in0=ident[:, 0:N // P * H],
                scalar1=0.0, scalar2=1.0, op0=ALU.mult, op1=ALU.add,
            )
            nc.vector.memset(v4[:, :, :, HD + 1:], 0.0)

            for _rep in range(reps):
                # ---------- load x first (critical path: GN stats) ----------
                x_sb = []
                for t in range(CT):
                    xt = pp.tile([P, N], BF16, name=f"x{t}", tag=f"x{t}")
                    for ch in range(2):
                        eng = nc.sync if (ch + t) % 2 == 0 else nc.gpsimd
                        eng.dma_start(
                            out=xt[:, ch * 2048:(ch + 1) * 2048],
                            in_=x_d[t * P:(t + 1) * P, ch * 2048:(ch + 1) * 2048],
                        )
                    x_sb.append(xt)

                # ---------- weight DMAs (wp defers under the stream) -------
                wT = {}  # wT[nm][ct] : [128 (in_c), 256 (out_c)] bf16
                wraw = {}
                for nm in ("wq", "wk", "wv", "wp"):
                    wT[nm] = [pp.tile([P, C], BF16, name=f"{nm}T{ct}", tag=f"{nm}T{ct}") for ct in range(CT)]
                for nm in ("wq", "wk", "wv"):
                    wr = wraw_pool.tile([P, CT, C], F32, name="wraw", tag="wraw")
                    weng = nc.gpsimd if nm == "wk" else nc.sync
                    weng.dma_start(
                        out=wr, in_=w_d[nm].rearrange("(t p) c -> p t c", t=CT)
                    )
                    wraw[nm] = wr

                def emit_wT(nm):
                    wr = wraw[nm]
                    for ot in range(CT):  # row tile of W = out-channel tile
                        for ct in range(CT):  # col block = in-channel block
                            ps_t = pm_pool.tile([P, P], F32, name="pm", tag="pm")
                            nc.tensor.transpose(
                                ps_t, wr[:, ot, ct * P:(ct + 1) * P], ident
                            )
                            nc.vector.tensor_copy(
                                out=wT[nm][ct][:, ot * P:(ot + 1) * P], in_=ps_t
                            )

                if stop_after == "load":
                    xres_sb = pp.tile([P, CT, NS], F32, name="xres", tag="xres")
                    nc.gpsimd.dma_start(
                        out=xres_sb, in_=xres_d.rearrange("(t p) c -> p t c", t=CT)
                    )
                    _write_trivial(nc, outp, out_d, xres_sb)
                    continue

                # ---------- GroupNorm: per-tile stats + aggregation chains --
                # tile 0's post-stats chain runs on GpSimd while the DVE
                # crunches tile 1's bn_stats; tile 1's chain runs on DVE.
                # A = rstd*gamma, B = beta - mean*A; GN folds into the QKV
                # weights as W <- W diag(A), bias += W@B.
                stat2_all = sm.tile([P, CT, 2], F32, name="mvall", tag="mvall")
                psum_g = pm_pool.tile([16, CT, 2], F32, name="pm", tag="pm")
                A_sb, B_sb, AQ_sb = [], [], []
                for t in range(CT):
                    stats = sm.tile([P, 8, 6], F32, name=f"bnst{t}", tag=f"bnst{t}")
                    for sg in range(8):
                        nc.vector.bn_stats(
                            out=stats[:, sg, :],
                            in_=x_sb[t][:, sg * 512:(sg + 1) * 512],
                        )
                    mv = stat2_all[:, t, :]
                    nc.vector.bn_aggr(out=mv, in_=stats)
                    nc.vector.scalar_tensor_tensor(
                        out=mv[:, 1:2], in0=mv[:, 0:1], scalar=mv[:, 0:1],
                        in1=mv[:, 1:2], op0=ALU.mult, op1=ALU.add,
                    )
                    e = nc.vector
                    nc.tensor.matmul(psum_g[:, t, :], mask8, mv,
                                     start=True, stop=True)
                    if t == 0:
                        # PE slack while chains run: transpose wq/wk
                        emit_wT("wq")
                        emit_wT("wk")
                    gmean_t = psum_g[:, t, 0:1]
                    gE2_t = psum_g[:, t, 1:2]
                    # PSUM-reading ops must stay on the DVE (GPSIMD cannot
                    # access PSUM); SBUF-only minis go to `e`.
                    gst = sm.tile([16, 2], F32, name=f"gst{t}", tag=f"gst{t}")
                    nc.vector.tensor_copy(out=gst[:, 0:1], in_=gmean_t)
                    veps = sm.tile([16, 1], F32, name=f"veps{t}", tag=f"veps{t}")
                    gmsq = sm.tile([16, 1], F32, name=f"gmsq{t}", tag=f"gmsq{t}")
                    e.tensor_mul(gmsq, gst[:, 0:1], gst[:, 0:1])
                    nc.vector.scalar_tensor_tensor(
                        out=veps, in0=gE2_t, scalar=EPS, in1=gmsq,
                        op0=ALU.add, op1=ALU.subtract,
                    )
                    # rstd = rsqrt(var+eps): bitcast-Newton (keeps the Act
                    # engine exp-only -> exactly one act-table load)
                    zi = sm.tile([16, 1], I32, name=f"zi{t}", tag=f"zi{t}")
                    e.tensor_scalar(
                        out=zi, in0=veps.bitcast(I32), scalar1=1, scalar2=None,
                        op0=ALU.logical_shift_right,
                    )
                    e.tensor_scalar(
                        out=zi, in0=zi, scalar1=-1, scalar2=RSQRT_MAGIC,
                        op0=ALU.mult, op1=ALU.add,
                    )
                    z = zi.bitcast(F32)
                    tmp_n = sm.tile([16, 1], F32, name=f"tmpn{t}", tag=f"tmpn{t}")
                    for it in range(2):
                        e.tensor_mul(tmp_n, z, z)
                        e.tensor_mul(tmp_n, tmp_n, veps)
                        e.tensor_scalar(
                            out=tmp_n, in0=tmp_n, scalar1=-0.5, scalar2=1.5,
                            op0=ALU.mult, op1=ALU.add,
                        )
                        e.tensor_mul(gst[:, 1:2] if it == 1 else z, z, tmp_n)
                    # broadcast group (mean, rstd) to this tile's channels;
                    # gamma/beta pairs host-packed in biasp[0] cols 3:5 / 5:7
                    psum_ch = pm_pool.tile([P, 2], F32, name="pm", tag="pm")
                    nc.tensor.matmul(psum_ch, mask16t, gst, start=True, stop=True)
                    A_t = sm.tile([P, 1], F32, name=f"A{t}", tag=f"A{t}")
                    nc.vector.tensor_mul(A_t, psum_ch[:, 1:2], biasp[0][:, 3 + t:4 + t])
                    tmp_c = sm.tile([P, 1], F32, name=f"mt{t}", tag=f"mt{t}")
                    nc.vector.tensor_mul(tmp_c, psum_ch[:, 0:1], A_t)
                    B_t = sm.tile([P, 1], BF16, name=f"B{t}", tag=f"B{t}")
                    e.tensor_sub(B_t, biasp[0][:, 5 + t:6 + t], tmp_c)
                    aq = sm.tile([P, 1], F32, name=f"AQ{t}", tag=f"AQ{t}")
                    e.tensor_scalar_mul(aq, A_t, SCALE)
                    A_sb.append(A_t)
                    B_sb.append(B_t)
                    AQ_sb.append(aq)

                # qb = (Wq@B + bq)*scale (unfolded wq; folds below wait).
                # K needs no bias: softmax is invariant to per-query shifts.
                qb = []
                for ot in range(CT):
                    psum_bc = pm_pool.tile([P, 1], F32, name="pm", tag="pm")
                    for ct in range(CT):
                        nc.tensor.matmul(
                            psum_bc,
                            wT["wq"][ct][:, ot * P:(ot + 1) * P],
                            B_sb[ct],
                            start=(ct == 0), stop=(ct == CT - 1),
                        )
                    b_t = sm.tile([P, 1], F32, name=f"bcq{ot}", tag=f"bcq{ot}")
                    nc.vector.tensor_scalar(
                        out=b_t, in0=psum_bc,
                        scalar1=bias_sb[("bq", ot)], scalar2=SCALE,
                        op0=ALU.add, op1=ALU.mult,
                    )
                    qb.append(b_t)
                # fold A (and hd^-0.5 for Q) into the weight columns
                for ct in range(CT):
                    nc.vector.tensor_scalar_mul(wT["wq"][ct], wT["wq"][ct], AQ_sb[ct])
                    nc.vector.tensor_scalar_mul(wT["wk"][ct], wT["wk"][ct], A_sb[ct])

                def emit_q(ot, pool=None, tg="po"):
                    # pre-stream: borrow the idle po pool so the long-lived Q
                    # psum stays out of pm's rotation (it would stall the K/V
                    # psums behind the q8 reads). In-stream (pass 1): pm is
                    # idle instead — po holds live O accumulators.
                    pool = pool or po_pool
                    psum_q = pool.tile([P, NS], F32, name=tg, tag=tg)
                    for ct in range(CT):
                        nc.tensor.matmul(
                            psum_q,
                            wT["wq"][ct][:, ot * P:(ot + 1) * P],
                            x_sb[ct][:, 0:NS],
                            start=(ct == 0), stop=(ct == CT - 1),
                        )
                    nc.vector.tensor_scalar(
                        out=q8e[ot][:, 0, :], in0=psum_q,
                        scalar1=qb[ot], scalar2=None, op0=ALU.add,
                    )
                    nc.vector.tensor_scalar(
                        out=q8o[ot][:, 1, :], in0=psum_q,
                        scalar1=qb[ot], scalar2=None, op0=ALU.add,
                    )

                # pass 0 needs only Q channels 0..127 (heads 0,1) — emit
                # those now; everything else defers under the exp stream.
                emit_q(0)
                emit_wT("wv")
                # wv_corr = Wv@B + bv must read UNFOLDED wv: emit before fold
                wv_corr = []
                for ot in range(CT):
                    psum_bc = pm_pool.tile([P, 1], F32, name="pm", tag="pm")
                    for ct in range(CT):
                        nc.tensor.matmul(
                            psum_bc,
                            wT["wv"][ct][:, ot * P:(ot + 1) * P],
                            B_sb[ct],
                            start=(ct == 0), stop=(ct == CT - 1),
                        )
                    b_t = sm.tile([P, 1], BF16, name=f"bcv{ot}", tag=f"bcv{ot}")
                    nc.vector.tensor_add(b_t, psum_bc, bias_sb[("bv", ot)])
                    wv_corr.append(b_t)
                for ct in range(CT):
                    nc.vector.tensor_scalar_mul(wT["wv"][ct], wT["wv"][ct], A_sb[ct])

                bpp = []
                xres_box = []

                # cheap queue ops now; transfers overlap the early stream
                wr_p = wraw_pool.tile([P, CT, C], F32, name="wraw", tag="wraw")
                nc.sync.dma_start(
                    out=wr_p, in_=w_d["wp"].rearrange("(t p) c -> p t c", t=CT)
                )
                wraw["wp"] = wr_p
                xres_sb = pp.tile([P, CT, NS], F32, name="xres", tag="xres")
                nc.gpsimd.dma_start(
                    out=xres_sb, in_=xres_d.rearrange("(t p) c -> p t c", t=CT)
                )
                xres_box.append(xres_sb)

                def emit_deferred():
                    # runs in engine slack once the exp stream is rolling
                    emit_wT("wp")
                    # proj bias absorbs the attention-output correction:
                    # bp' = bp + Wp @ wv_corr  (attn stores only O/denom)
                    for ot in range(CT):
                        psum_bp = pm_pool.tile([P, 1], F32, name="pm", tag="pm")
                        for ct in range(CT):
                            nc.tensor.matmul(
                                psum_bp,
                                wT["wp"][ct][:, ot * P:(ot + 1) * P],
                                wv_corr[ct],
                                start=(ct == 0), stop=(ct == CT - 1),
                            )
                        b_t = sm.tile([P, 1], F32, name=f"bpp{ot}", tag=f"bpp{ot}")
                        nc.vector.tensor_add(b_t, psum_bp, bias_sb[("bp", ot)])
                        bpp.append(b_t)

                if stop_after in ("gn", "conv"):
                    emit_deferred()
                    _write_trivial(nc, outp, out_d, xres_box[0])
                    continue

                # ---------- interleaved K/V production + attention ----------
                # pass 0: K/V blocks just-in-time + attention heads 0,1
                # pass 1: attention heads 2,3 (K/V already resident)
                attn_h = [pp.tile([P, NS], F32, name=f"attnh{i}", tag=f"attnh{i}") for i in range(2)]
                attnT = [pp.tile([P, 2, C], BF16, name=f"attnT{b}", tag=f"attnT{b}") for b in range(CT)]
                NJP = N // (SB * P)  # 16 pt tiles per head

                def emit_k(kjp):
                    # tokens [kjp*256, (kjp+1)*256) = key blocks 2kjp, 2kjp+1
                    for ot in range(CT):
                        psum_k = pm_pool.tile([P, SB * P], F32, name="pm", tag="pm")
                        for ct in range(CT):
                            nc.tensor.matmul(
                                psum_k,
                                wT["wk"][ct][:, ot * P:(ot + 1) * P],
                                x_sb[ct][:, kjp * 256:(kjp + 1) * 256],
                                start=(ct == 0), stop=(ct == CT - 1),
                            )
                        nc.vector.tensor_copy(
                            out=k8[ot][:, :, kjp * P:(kjp + 1) * P],
                            in_=psum_k.rearrange("p (s c) -> p s c", s=2),
                        )

                def emit_o(psum_o, pt, jp, h):
                    nc.tensor.matmul(
                        psum_o[0:HDP, :],
                        v8[:, SB * jp:SB * (jp + 1), h * HDP:(h + 1) * HDP],
                        pt[:, :, :],
                        start=(jp == 0), stop=(jp == NJP - 1),
                        perf_mode=DR,
                    )

                def emit_sx(pas, jp):
                    # S + exp for both heads of this pass at key-block pair jp
                    pts = {}
                    for h in ((0, 1) if pas == 0 else (2, 3)):
                        r0 = (h % 2) * HD
                        psum_s = ps_pool.tile([P, SB, NS], F32, name="ps", tag="ps")
                        for b in range(SB):
                            qx = q8e if b == 0 else q8o
                            nc.tensor.matmul(
                                psum_s[:, b, :],
                                k8[pas][r0:r0 + HD, :, jp * P:(jp + 1) * P],
                                qx[pas][r0:r0 + HD, :, :],
                                start=True, stop=True,
                                perf_mode=DR,
                            )
                        pt = pt_pool.tile([P, SB, NS], F8, name="pt", tag="pt")
                        nc.scalar.activation(out=pt, in_=psum_s, func=AF.Exp,
                                             bias=nbias)
                        pts[h] = pt
                    return pts

                def emit_v(jp):
                    for b in range(SB):
                        jt = jp * SB + b
                        psum_v = pm_pool.tile([P, C], F32, name="pm", tag="pm")
                        for ct in range(CT):
                            nc.tensor.matmul(
                                psum_v,
                                x_sb[ct][:, jt * P:(jt + 1) * P],
                                wT["wv"][ct],
                                start=(ct == 0), stop=(ct == CT - 1),
                            )
                        nc.vector.tensor_copy(
                            out=v4[:, jt, :, 0:HD],
                            in_=psum_v.rearrange("p (h d) -> p h d", d=HD),
                        )

                def emit_completion(pas, h, po_h, pend):
                    for ojp, opt in pend[h]:
                        emit_o(po_h[h], opt, ojp, h)
                    pend[h] = []
                    r0 = (h % 2) * HD
                    # stash unnormalized O + denom row; normalization happens
                    # after the transpose, where the per-token denominator
                    # becomes a per-partition scalar
                    ah = attn_h[h % 2]
                    nc.vector.tensor_copy(out=ah[0:HD + 1, :],
                                          in_=po_h[h][0:HD + 1, :])
                    if stop_after is None:
                        # per-head un-reshape half-transposes: head h's 64
                        # attn channels -> attnT columns
                        for s in range(2):
                            for b in range(CT):
                                ps_t = pm_pool.tile([P, HD + 1], F32, name="pm", tag="pm")
                                nc.tensor.transpose(
                                    ps_t,
                                    ah[0:HD + 1,
                                       s * 256 + b * P:s * 256 + (b + 1) * P],
                                    ident[0:HD + 1, 0:HD + 1],
                                )
                                rd = sm.tile([P, 1], F32, name="rd", tag="rd")
                                nc.vector.reciprocal(out=rd,
                                                     in_=ps_t[:, HD:HD + 1])
                                nc.vector.tensor_scalar(
                                    out=attnT[b][:, s, pas * P + r0:pas * P + r0 + HD],
                                    in0=ps_t[:, 0:HD],
                                    scalar1=rd, scalar2=None, op0=ALU.mult,
                                )

                # ---- pass 0: heads 0,1 with JIT K/V production ----
                po_h0 = {h: po_pool.tile([P, NS], F32, name="po", tag="po")
                         for h in (0, 1)}
                pend0 = {0: [], 1: []}
                emit_k(0)
                emit_q(1)  # PE slack while waiting on k8/q8 block 0
                for jp in range(NJP):
                    pts = emit_sx(0, jp)
                    if jp + 1 < NJP:
                        emit_k(jp + 1)
                    emit_v(jp)
                    for h in (0, 1):
                        pend0[h].append((jp, pts[h]))
                        if len(pend0[h]) > SKEW:
                            ojp, opt = pend0[h].pop(0)
                            emit_o(po_h0[h], opt, ojp, h)

                # ---- pass-1 prefetch: keep the Act engine fed while
                # pass-0's completion chain drains ----
                po_h1 = {h: po_pool.tile([P, NS], F32, name="po", tag="po")
                         for h in (2, 3)}
                pend1 = {2: [], 3: []}
                pts = emit_sx(1, 0)
                for h in (2, 3):
                    pend1[h].append((0, pts[h]))

                # ---- pass-0 completion + deferred fill ----
                for h in (0, 1):
                    emit_completion(0, h, po_h0, pend0)
                emit_deferred()

                # ---- pass 1: heads 2,3 (K/V resident) ----
                for jp in range(1, NJP):
                    pts = emit_sx(1, jp)
                    for h in (2, 3):
                        pend1[h].append((jp, pts[h]))
                        if len(pend1[h]) > SKEW:
                            ojp, opt = pend1[h].pop(0)
                            emit_o(po_h1[h], opt, ojp, h)
                for h in (2, 3):
                    emit_completion(1, h, po_h1, pend1)

                if stop_after == "attn":
                    _write_trivial(nc, outp, out_d, xres_box[0])
                    continue
                # ---------- proj + bias + residual ----------
                for s in range(2):
                    for ot in range(CT):
                        psum_y = pm_pool.tile([P, C], F32, name="pm", tag="pm")
                        for ct in range(CT):
                            nc.tensor.matmul(
                                psum_y,
                                wT["wp"][ct][:, ot * P:(ot + 1) * P],
                                attnT[ct][:, s, :],
                                start=(ct == 0), stop=(ct == CT - 1),
                            )
                        y_sb = outp.tile([P, C], F32, name="ysb", tag="ysb")
                        c0 = s * 256
                        nc.vector.scalar_tensor_tensor(
                            out=y_sb, in0=psum_y,
                            scalar=bpp[ot],
                            in1=xres_box[0][:, ot, c0:c0 + C],
                            op0=ALU.add, op1=ALU.add,
                        )
                        oeng = nc.sync if ot == 0 else nc.gpsimd
                        oeng.dma_start(
                            out=out_d[ot * P:(ot + 1) * P, c0:c0 + C],
                            in_=y_sb,
                        )
    nc.compile()
    return nc


def _host_constants():
    ident = np.eye(P, dtype=np.float32)
    mask8 = np.zeros((P, 16), dtype=np.float32)
    mask8[np.arange(P), np.arange(P) // GPC] = 1.0 / GPC
    mask16t = np.zeros((16, P), dtype=np.float32)
    mask16t[np.arange(P) // GPC, np.arange(P)] = 1.0
    return ident, mask8, mask16t


def make_in_maps(x_kv, gn_gamma, gn_beta, Wq, bq, Wk, bk, Wv, bv, Wp, bp):
    x2 = np.ascontiguousarray(np.asarray(x_kv, dtype=np.float32).reshape(C, N))
    ident, mask8, mask16t = _host_constants()

    biasp = np.zeros((C, 8), dtype=np.float32)
    biasp[:, 0] = np.asarray(bq, np.float32)
    biasp[:, 1] = np.asarray(bv, np.float32)
    biasp[:, 2] = np.asarray(bp, np.float32)
    gam = np.asarray(gn_gamma, np.float32)
    bet = np.asarray(gn_beta, np.float32)
    biasp[:P, 3] = gam[:P]
    biasp[:P, 4] = gam[P:]
    biasp[:P, 5] = bet[:P]
    biasp[:P, 6] = bet[P:]

    common = {
        "wq": np.ascontiguousarray(np.asarray(Wq, np.float32)),
        "wk": np.ascontiguousarray(np.asarray(Wk, np.float32)),
        "wv": np.ascontiguousarray(np.asarray(Wv, np.float32)),
        "wp": np.ascontiguousarray(np.asarray(Wp, np.float32)),
        "biasp": biasp,
        "ident": ident,
        "mask8": mask8,
        "mask16t": mask16t,
    }

    in_maps = []
    for cid in range(NCORES):
        own = np.concatenate(
            [np.arange(2 * cid, N, 16), np.arange(2 * cid + 1, N, 16)]
        )
        rest = np.setdiff1d(np.arange(N), own)
        perm = np.concatenate([own, rest])
        m = dict(common)
        m["x"] = np.ascontiguousarray(
            x2[:, perm].astype(ml_dtypes.bfloat16)
        )
        m["xres"] = np.ascontiguousarray(x2[:, NS * cid:NS * (cid + 1)])
        in_maps.append(m)
    return in_maps


def kernel(x_kv, gn_gamma, gn_beta, Wq, bq, Wk, bk, Wv, bv, Wp, bp, **run_kwargs):
    if "nc" not in _CACHE:
        _CACHE["nc"] = build_nc()
    nc = _CACHE["nc"]

    in_maps = make_in_maps(
        x_kv, gn_gamma, gn_beta, Wq, bq, Wk, bk, Wv, bv, Wp, bp
    )

    res = run_bass_kernel_spmd(
        nc, in_maps, core_ids=list(range(NCORES)), **run_kwargs
    )
    y = np.empty((C, N), dtype=np.float32)
    for cid in range(NCORES):
        y[:, NS * cid:NS * (cid + 1)] = res.results[cid]["out"]
    _CACHE["last_results"] = res
    return y.reshape(1, C, 64, 64)
